# revision 34
# baseline (speedup 1.0000x reference)
"""CLAHE (nn_EqualizeClahe) Trainium2 Bass kernel, v2.

kernel(x): x (8,3,1024,1024) fp32 in [0,1) -> same-shape output.
8 NeuronCores data parallel: core i processes image i (3 channels).

Per channel (1024x1024, 8x8 grid of 128x128 tiles):
  prep:  u    = round(x*256 + 0.5)  (= bin+1, exact for x = k*2^-24)
         uidx = round(x*255 + 0.5)  (= lut index + 1)
  hist:  per tile, 256 bins as a 16x16 (hi x lo) outer product accumulated
         on the TensorEngine, one column-matmul per pixel column; hi/lo
         one-hots built by DVE is_equal slabs in bf16.
  lut:   hist flattened to [tile(64 partitions), 256 bins]; clip at 2560,
         cumsum by shift-add doubling along the free dim, uniform
         redistribution of the clipped excess, floored scale to final
         LUT/255 in bf16.
  apply: ONE indirect_copy per half-band: each GPSIMD core's 16 partitions
         hold the 16 neighbour-tile LUTs (2 tile rows x 8 tile cols) its
         16 pixel rows may reference; the raw uidx tensor is the index
         stream, so every pixel is looked up through all 16 LUTs at once.
         A static wx-weight multiply (DVE), then 16 small matmuls whose
         lhsT constants carry the wy weights reduce each core's 16
         partitions into the final blended pixel, stacked 3 k-groups per
         PSUM tile (bases 0/32/64).  Two [128,512] copies per psum tile
         evacuate to SBUF and one strided DMA per (band, k-mod-3 group)
         writes DRAM in row order.

Self-contained: only needs /opt/trn_rl_repo (concourse) + numpy.
"""
import sys

for _p in ("/opt/trn_rl_repo",):
    if _p not in sys.path:
        sys.path.insert(0, _p)

import dataclasses
from contextlib import ExitStack

import numpy as np

import concourse.bass as bass
import concourse.mybir as mybir
import concourse.tile as tile
from concourse.bass_utils import run_bass_kernel_spmd

FP32 = mybir.dt.float32
BF16 = mybir.dt.bfloat16
U16 = mybir.dt.uint16
U8 = mybir.dt.uint8
OP = mybir.AluOpType
ACT = mybir.ActivationFunctionType

H = W = 1024
CH = 3
NB = 256
TS = 128          # tile size (8x8 grid)
MAXV = 2560.0     # clip limit * pixels / bins
LUT_SCALE = float(np.float32(255.0 / 16384.0))

_CACHE = {}


def _bcast_free(ap, n):
    """[P, 1] -> [P, n] via a step-0 free dim."""
    new = [ap.ap[0], [0, n]]
    return dataclasses.replace(ap, ap=new)


def _interp_coords(n_tiles, tile_size, length):
    half = tile_size // 2
    pos = np.arange(length)
    j = pos // half
    p = pos % half
    r0 = np.clip((j - 1) // 2, 0, n_tiles - 1)
    r1 = np.clip(r0 + 1, 0, n_tiles - 1)
    denom = np.float32(2 * half - 1)
    w = np.where(j % 2 == 1, (2 * half - 1) - p, (half - 1) - p).astype(np.float32) / denom
    w = np.where(j == 0, np.float32(1.0), w).astype(np.float32)
    return r0, r1, w


def _host_consts():
    import ml_dtypes
    c0, c1, wx = _interp_coords(8, TS, W)
    r0, r1, wy = _interp_coords(8, TS, H)

    # WX[p, 16*x + k] = wx-role weight of tile-col (p%8) at column x
    wxrow = np.zeros((8, W), np.float32)
    for cp in range(8):
        wxrow[cp] = wx * (c0 == cp) + (1.0 - wx) * (c1 == cp)
    wx_full = np.zeros((128, W, 16), np.float32)
    for p in range(128):
        wx_full[p, :, :] = wxrow[p % 8][:, None]
    wx_full = wx_full.reshape(128, 16 * W)

    # L[p, (a*16+k)*8 + c]: wy-weighted core-reduction lhsT
    lab = np.zeros((128, 8 * 16 * 8), np.float32)
    for a in range(8):
        for k in range(16):
            for c in range(8):
                y = 128 * a + 16 * c + k
                for p in range(16 * c, 16 * c + 16):
                    s = (p % 16) // 8
                    lab[p, (a * 16 + k) * 8 + c] = wy[y] if s == 0 else 1.0 - wy[y]

    iota1 = np.broadcast_to(np.arange(1, NB + 1, dtype=np.float32)[None, :],
                            (64, NB)).copy()
    return {
        "wx": wx_full.astype(ml_dtypes.bfloat16),
        "lab": lab.astype(ml_dtypes.bfloat16),
        "iota1": iota1,
    }


# ----------------------------------------------------------------- kernel IR
def _emit(nc, tc, ctx, x_in, y_out, K):
    pool = ctx.enter_context(tc.tile_pool(name="main", bufs=1))
    pool2 = ctx.enter_context(tc.tile_pool(name="dbuf", bufs=2))
    pool4 = ctx.enter_context(tc.tile_pool(name="qbuf", bufs=3))
    pshist = ctx.enter_context(tc.tile_pool(name="pshist", bufs=2, space="PSUM"))
    psred = ctx.enter_context(tc.tile_pool(name="psred", bufs=1, space="PSUM"))

    wxb = pool.tile([128, 16 * W], BF16, tag="wxb")
    nc.sync.dma_start(wxb[:], K["wx"].ap())
    lab = pool.tile([128, 1024], BF16, tag="lab")
    nc.sync.dma_start(lab[:], K["lab"].ap())
    iot = pool.tile([64, NB], FP32, tag="iot")
    nc.sync.dma_start(iot[:], K["iota1"].ap())

    def hist_prep(ch, a, uidx):
        """Load band, produce hbf (hi one-hot input), lo (lo+1), store uidx u8."""
        xb = pool2.tile([128, W], FP32, tag="xb", name=f"xb_{ch}_{a}")
        nc.sync.dma_start(xb[:], x_in[ch, a * 128:(a + 1) * 128, :])
        u1 = pool2.tile([128, W], U16, tag="u1", name=f"u1_{ch}_{a}")
        nc.scalar.activation(u1[:], xb[:], ACT.Copy, bias=0.5, scale=256.0)
        nc.scalar.activation(uidx[:, a * W:(a + 1) * W], xb[:], ACT.Copy,
                             bias=0.5 + 2.0 ** -12, scale=255.0)
        ubf = pool2.tile([128, W], BF16, tag="ubf", name=f"ubf_{ch}_{a}")
        nc.vector.tensor_copy(ubf[:], u1[:])
        hiu = pool2.tile([128, W], U16, tag="hiu", name=f"hiu_{ch}_{a}")
        nc.scalar.activation(hiu[:], ubf[:], ACT.Copy, bias=-0.5525, scale=0.0625)
        hbf = pool2.tile([128, W], BF16, tag="hbf", name=f"hbf_{ch}_{a}")
        nc.vector.tensor_copy(hbf[:], hiu[:])
        lo = pool2.tile([128, W], BF16, tag="lo", name=f"lo_{ch}_{a}")
        nc.vector.scalar_tensor_tensor(lo[:], hbf[:], -16.0, ubf[:],
                                       op0=OP.mult, op1=OP.add)  # lo+1 in 1..16
        return hbf, lo

    def hist_quarter(ch, a, qd, hbf, lo, hp):
        # half-band slabs (qd = 0, 2 start a half; qd = 1, 3 run its 2nd tile pair)
        if qd % 2 == 0:
            h = qd // 2
            ohh = pool.tile([128, 16 * 512], BF16, tag="ohh")
            ohl = pool.tile([128, 16 * 512], BF16, tag="ohl")
            for j in range(16):
                nc.vector.tensor_scalar(ohh[:, j * 512:(j + 1) * 512],
                                        hbf[:, h * 512:(h + 1) * 512],
                                        float(j), None, op0=OP.is_equal)
                nc.vector.tensor_scalar(ohl[:, j * 512:(j + 1) * 512],
                                        lo[:, h * 512:(h + 1) * 512],
                                        float(j + 1), None, op0=OP.is_equal)
            hist_quarter.slabs = (ohh, ohl)
        ohh, ohl = hist_quarter.slabs
        oh3 = ohh[:].rearrange("p (j x) -> p j x", j=16)
        ol3 = ohl[:].rearrange("p (j x) -> p j x", j=16)
        for t in range(2):
            T = 2 * qd + t
            xt = (2 * qd + t) % 4
            for col in range(128):
                xl = 128 * xt + col
                nc.tensor.matmul(hp[:, 16 * T:16 * T + 16],
                                 oh3[:, :, xl], ol3[:, :, xl],
                                 start=(col == 0), stop=(col == 127))

    def hsb_flush(ch, a, hp, hsb, ht):
        nc.scalar.copy(hsb[:, a * 128:(a + 1) * 128], hp[:])
        for tt_ in range(8):
            t = 8 * a + tt_
            nc.sync.dma_start(ht[t:t + 1, :],
                              hsb[0:16, a * 128 + 16 * tt_: a * 128 + 16 * tt_ + 16])

    def lut_build(ch, hsb, ht):
        ca = pool.tile([64, NB], FP32, tag="ca")
        cb = pool.tile([64, NB], FP32, tag="cb")
        nc.vector.tensor_scalar(ca[:], ht[:], MAXV, None, op0=OP.min)
        cur, nxt = ca, cb
        for s in (1, 2, 4, 8, 16, 32, 64, 128):
            nc.vector.tensor_copy(nxt[:, :s], cur[:, :s])
            nc.vector.tensor_tensor(nxt[:, s:], cur[:, s:], cur[:, :NB - s], op=OP.add)
            cur, nxt = nxt, cur
        # cur = cumsum C; redistribution
        clip16 = pool.tile([64, 1], FP32, tag="clip16")
        nc.vector.tensor_scalar(clip16[:], cur[:, NB - 1:NB], -1.0, 16384.0,
                                op0=OP.mult, op1=OP.add)
        bsf = pool.tile([64, 1], FP32, tag="bsf")
        nc.vector.tensor_scalar(bsf[:], clip16[:], 1.0 / 256.0, -0.49,
                                op0=OP.mult, op1=OP.add)
        bsu = pool.tile([64, 1], U16, tag="bsu")
        nc.scalar.copy(bsu[:], bsf[:])
        nc.scalar.copy(bsf[:], bsu[:])  # base, exact fp32
        resid = pool.tile([64, 1], FP32, tag="resid")
        nc.vector.scalar_tensor_tensor(resid[:], bsf[:], -256.0, clip16[:],
                                       op0=OP.mult, op1=OP.add)
        tmp = pool.tile([64, NB], FP32, tag="tmp")
        nc.vector.tensor_tensor(tmp[:], iot[:], _bcast_free(bsf[:], NB), op=OP.mult)
        nc.vector.tensor_tensor(nxt[:], cur[:], tmp[:], op=OP.add)
        nc.vector.tensor_tensor(tmp[:], iot[:], _bcast_free(resid[:], NB), op=OP.min)
        nc.vector.tensor_tensor(cur[:], nxt[:], tmp[:], op=OP.add)
        # LUT = floor(C~ * 255/16384) / 255, fp32
        nc.vector.tensor_scalar(nxt[:], cur[:], LUT_SCALE, -0.499,
                                op0=OP.mult, op1=OP.add)
        lu16 = pool.tile([64, NB], U16, tag="lu16")
        nc.scalar.copy(lu16[:], nxt[:])
        ltb0 = pool.tile([64, NB], FP32, tag="ltb0")
        nc.scalar.copy(ltb0[:], lu16[:])
        lt = pool.tile([64, NB], FP32, tag="lt")
        nc.vector.tensor_scalar(lt[:], ltb0[:], float(np.float32(1.0) / np.float32(255.0)),
                                None, op0=OP.mult)
        # TB [128, 8*257]: per band a, partition 16c+8s+c' holds the LUT of
        # tile (rowpair(a, c//4, s), c'), entries shifted by one (T[u]=LUT[u-1];
        # entry 0 is never read since u >= 1).
        tb = pool.tile([128, 8 * 257], FP32, tag="tb")
        for a in range(8):
            off = 257 * a
            rtop = max(a - 1, 0)
            rbot = a
            if a >= 1:
                nc.sync.dma_start(tb[0:16, off + 1: off + 257], lt[8 * rtop: 8 * rtop + 16, :])
            else:
                for s in range(2):
                    nc.sync.dma_start(tb[8 * s:8 * s + 8, off + 1: off + 257], lt[0:8, :])
            if a <= 6:
                nc.sync.dma_start(tb[64:80, off + 1: off + 257], lt[8 * rbot: 8 * rbot + 16, :])
            else:
                for s in range(2):
                    nc.sync.dma_start(tb[64 + 8 * s: 72 + 8 * s, off + 1: off + 257], lt[56:64, :])
        nc.sync.dma_start(tb[16:32, :], tb[0:16, :])
        nc.sync.dma_start(tb[32:64, :], tb[0:32, :])
        nc.sync.dma_start(tb[80:96, :], tb[64:80, :])
        nc.sync.dma_start(tb[96:128, :], tb[64:96, :])
        return tb

    def apply_stage_idx(ch, a, uidx):
        ustg = pool2.tile([128, W], U16, tag="ustg", name=f"ustg_{ch}_{a}")
        nc.scalar.copy(ustg[:], uidx[:, a * W:(a + 1) * W])
        return ustg

    def apply_gather_half(ch, a, tb, ustg, h, ndve=4):
        """8 chunked gathers (Pool); first 8-ndve converts on Act, rest deferred."""
        gb = pool2.tile([128, 8192], BF16, tag="gb", name=f"gb_{ch}_{a}_{h}")
        gfs = []
        for cc in range(8):
            gf = pool4.tile([128, 1024], FP32, tag="gf", name=f"gf_{ch}_{a}_{h}_{cc}")
            nc.gpsimd.indirect_copy(
                gf[:], tb[:, 257 * a: 257 * a + 257],
                ustg[:, 512 * h + 64 * cc: 512 * h + 64 * cc + 64], True)
            if cc < 8 - ndve:
                nc.scalar.copy(gb[:, 1024 * cc: 1024 * cc + 1024], gf[:])
            else:
                gfs.append((cc, gf))
        return gb, gfs

    def apply_mult_half(ch, a, gb, gfs, h):
        for cc, gf in gfs:
            nc.vector.tensor_copy(gb[:, 1024 * cc: 1024 * cc + 1024], gf[:])
        nc.vector.tensor_tensor(gb[:, :4096], gb[:, :4096],
                                wxb[:, 8192 * h: 8192 * h + 4096], op=OP.mult)
        nc.vector.tensor_tensor(gb[:, 4096:], gb[:, 4096:],
                                wxb[:, 8192 * h + 4096: 8192 * h + 8192], op=OP.mult)
        g3 = gb[:].rearrange("p (x k) -> p x k", k=16)
        pt = []
        for T in range(2):
            t = psred.tile([128, 1536], FP32, tag=f"rT{T}", name=f"rT{T}_{ch}_{a}_{h}")
            pt.append(t)
        for k in range(16):
            q, m = k // 3, k % 3
            T, s = q // 3, q % 3
            nc.tensor.matmul(pt[T][32 * m:32 * m + 8, 512 * s: 512 * s + 512],
                             lab[:, (a * 16 + k) * 8: (a * 16 + k) * 8 + 8],
                             g3[:, :, k], start=True, stop=True)
        return pt

    def apply_out_half(ch, a, pt, h, dve_evac=False):
        f3 = pool2.tile([128, 6 * 512], FP32, tag="f3", name=f"f3_{ch}_{a}_{h}")
        for T in range(2):
            if dve_evac:
                nc.vector.tensor_copy(f3[:, T * 1536: (T + 1) * 1536], pt[T][:])
            else:
                nc.scalar.copy(f3[:, T * 1536: (T + 1) * 1536], pt[T][:])
        for m in range(3):
            nq = 6 if m == 0 else 5
            s0 = f3[32 * m:32 * m + 8, :]
            src2 = dataclasses.replace(s0, ap=[s0.ap[0], [512, nq], [1, 512]])
            dst = dataclasses.replace(
                y_out,
                offset=y_out.offset + (ch * H + 128 * a + m) * W + 512 * h,
                ap=[[16 * W, 8], [3 * W, nq], [1, 512]])
            nc.sync.dma_start(dst, src2)

    def hist_band(ch, a, uidx, hsb, ht):
        hbf, lo = hist_prep(ch, a, uidx)
        hp = pshist.tile([16, 128], FP32, tag="hp")
        for qd in range(4):
            hist_quarter(ch, a, qd, hbf, lo, hp)
        hsb_flush(ch, a, hp, hsb, ht)

    def fused_band(ch_a, a, uidx_a, tb_a, ch_h, h_a, uidx_h, hsb_h, ht_h, preps, ustgs):
        """apply(ch_a, a) interleaved with hist(ch_h, h_a), stage-level order."""
        if a not in ustgs:
            ustgs[a] = apply_stage_idx(ch_a, a, uidx_a)
        ustg = ustgs.pop(a)
        if h_a is not None:
            if h_a not in preps:
                preps[h_a] = hist_prep(ch_h, h_a, uidx_h)
            hbf, lo = preps.pop(h_a)
            hp = pshist.tile([16, 128], FP32, tag="hp")
        gb0, gfs0 = apply_gather_half(ch_a, a, tb_a, ustg, 0)
        if h_a is not None:
            hist_quarter(ch_h, h_a, 0, hbf, lo, hp)
        gb1, gfs1 = apply_gather_half(ch_a, a, tb_a, ustg, 1)
        if h_a is not None:
            hist_quarter(ch_h, h_a, 1, hbf, lo, hp)
            if h_a + 1 < 8:
                preps[h_a + 1] = hist_prep(ch_h, h_a + 1, uidx_h)
        if a + 1 < 8:
            ustgs[a + 1] = apply_stage_idx(ch_a, a + 1, uidx_a)
        if h_a is not None:
            hist_quarter(ch_h, h_a, 2, hbf, lo, hp)
        pt0 = apply_mult_half(ch_a, a, gb0, gfs0, 0)
        if h_a is not None:
            hist_quarter(ch_h, h_a, 3, hbf, lo, hp)
        apply_out_half(ch_a, a, pt0, 0)
        pt1 = apply_mult_half(ch_a, a, gb1, gfs1, 1)
        if h_a is not None:
            hsb_flush(ch_h, h_a, hp, hsb_h, ht_h)
        apply_out_half(ch_a, a, pt1, 1)

    def apply_drain(ch, uidx, tb):
        ustgs = {0: apply_stage_idx(ch, 0, uidx)}
        for a in range(8):
            ustg = ustgs.pop(a)
            gb0, gfs0 = apply_gather_half(ch, a, tb, ustg, 0, ndve=4)
            gb1, gfs1 = apply_gather_half(ch, a, tb, ustg, 1, ndve=4)
            if a + 1 < 8:
                ustgs[a + 1] = apply_stage_idx(ch, a + 1, uidx)
            pt0 = apply_mult_half(ch, a, gb0, gfs0, 0)
            apply_out_half(ch, a, pt0, 0)
            pt1 = apply_mult_half(ch, a, gb1, gfs1, 1)
            apply_out_half(ch, a, pt1, 1)

    # ---- pipeline: hist(ch) -> lut(ch) -> {apply(ch) || hist(ch+1) one band ahead} ----
    cur_u = pool2.tile([128, 8 * W], U8, tag="uidx", name="uidx0")
    cur_s = pool2.tile([16, 8 * 128], FP32, tag="hsb", name="hsb0")
    cur_t = pool2.tile([64, NB], FP32, tag="ht", name="ht0")
    fill_preps = {0: hist_prep(0, 0, cur_u)}
    for a in range(8):
        hbf, lo = fill_preps.pop(a)
        hp = pshist.tile([16, 128], FP32, tag="hp")
        hist_quarter(0, a, 0, hbf, lo, hp)
        hist_quarter(0, a, 1, hbf, lo, hp)
        if a + 1 < 8:
            fill_preps[a + 1] = hist_prep(0, a + 1, cur_u)
        hist_quarter(0, a, 2, hbf, lo, hp)
        hist_quarter(0, a, 3, hbf, lo, hp)
        hsb_flush(0, a, hp, cur_s, cur_t)
    cur_tb = lut_build(0, cur_s, cur_t)
    for ch in range(CH):
        if ch + 1 < CH:
            nxt_u = pool2.tile([128, 8 * W], U8, tag="uidx", name=f"uidx{ch + 1}")
            nxt_s = pool2.tile([16, 8 * 128], FP32, tag="hsb", name=f"hsb{ch + 1}")
            nxt_t = pool2.tile([64, NB], FP32, tag="ht", name=f"ht{ch + 1}")
            preps = {}
            ustgs = {}
            hist_band(ch + 1, 0, nxt_u, nxt_s, nxt_t)
            nxt_tb = None
            for a in range(8):
                fused_band(ch, a, cur_u, cur_tb, ch + 1,
                           a + 1 if a < 7 else None, nxt_u, nxt_s, nxt_t,
                           preps, ustgs)
                if a == 6:
                    nxt_tb = lut_build(ch + 1, nxt_s, nxt_t)
            cur_tb = nxt_tb
            cur_u = nxt_u
        else:
            apply_drain(ch, cur_u, cur_tb)


def _apply_tile_patch():
    """This walrus build rejects >2 sync waits on one instruction; split the
    TileContext exit drain's waits into individual nops."""
    def _patched(self, tick_clock, wait_clock):
        nc = self.nc
        probe = nc.sync.nop()
        wait_clock.add_sem_waits(probe.ins,
                                 tile.ScopedClock({None: tick_clock.global_clock}))
        si = probe.ins.sync_info
        waits = list(si.on_wait) if si and si.on_wait else []
        if len(waits) > 1:
            probe.ins.sync_info = mybir.SyncInfo(on_wait=[waits[0]], on_update=[])
            for w in waits[1:]:
                extra = nc.sync.nop()
                extra.ins.sync_info = mybir.SyncInfo(on_wait=[w], on_update=[])
        nc.sync.drain()
        nc.all_engine_barrier()
        assert self.sems is not None
        popped = nc._tile_sem_poison_stack.pop()
        assert popped is self._sem_poison
        nc.clear_and_free_semaphores(list(self.sems.allocated().values()))
        nc.all_engine_barrier()
    tile.TileContext._drain_and_barrier = _patched


def _split_waits(nc, maxw=1):
    """This container's walrus rejects instructions with more than ~2 sem
    waits; hoist excess waits onto same-engine NoOps inserted just before."""
    import bass_rust
    counter = [0]
    for f in nc.m.functions:
        for blk in f.blocks:
            insts = blk.instructions
            out = []
            for ins in insts:
                si = ins.sync_info
                waits = list(si.on_wait) if si and si.on_wait else []
                if len(waits) > maxw:
                    keep = waits[:maxw]
                    extra = waits[maxw:]
                    for w in extra:
                        counter[0] += 1
                        nop = bass_rust.InstNoOp(
                            name=f"WSPLIT-{counter[0]}", engine=ins.engine,
                            ins=[], outs=[],
                            sync_info=mybir.SyncInfo(on_wait=[w], on_update=[]))
                        out.append(nop)
                    ins.sync_info = mybir.SyncInfo(
                        on_wait=keep, on_update=list(si.on_update or []))
                out.append(ins)
            blk.instructions = out


def build():
    if "nc" in _CACHE:
        return _CACHE["nc"]
    _apply_tile_patch()
    nc = bass.Bass("TRN2", target_bir_lowering=False, debug=False)
    x_in = nc.dram_tensor("x", [CH, H, W], FP32, kind="ExternalInput").ap()
    y_out = nc.dram_tensor("y", [CH, H, W], FP32, kind="ExternalOutput").ap()
    hk = _host_consts()
    K = {k: nc.inline_tensor(v, name=f"const_{k}") for k, v in hk.items()}
    with ExitStack() as ctx:
        tc = ctx.enter_context(tile.TileContext(nc))
        _emit(nc, tc, ctx, x_in, y_out, K)
    _split_waits(nc)
    _CACHE["nc"] = nc
    return nc


def kernel(x: np.ndarray) -> np.ndarray:
    x = np.ascontiguousarray(np.asarray(x, dtype=np.float32))
    assert x.shape == (8, CH, H, W), x.shape
    nc = build()
    in_maps = [{"x": x[i]} for i in range(8)]
    res = run_bass_kernel_spmd(nc, in_maps, list(range(8)))
    out = np.stack([res.results[i]["y"] for i in range(8)], axis=0)
    return out.astype(np.float32)


if __name__ == "__main__":
    x = np.random.rand(8, CH, H, W).astype(np.float32)
    y = kernel(x)
    print("ran:", y.shape, y.dtype)
